# revision 1
# baseline (speedup 1.0000x reference)
"""Trainium2 Bass kernel for nn_CrossAttentionLayer (m=n=1024, d=2048).

Math:  f = relu(term1 + term23 + term4 + ffn_b), where with
W1..W4 = ffn_w.reshape(n, 4, d) per-candidate blocks:
  term1  = sum_i u_p[i] . W1[i]
  term23 = <softmax_rows(S),    (W2 + u_p*W3) @ u_c.T>_F     (S = [m,n] logits)
  term4  = <softmax_rows(S.T),  (u_c*W4)      @ u_p.T>_F
Row-constant offsets cancel inside row-softmax, and the remaining column
offset folds into the matmul:  softmax_k(S[i,:]) = softmax_k((u_p[i]*w3 + w2) @ u_c.T).

Both inner products have the identical SPMD shape
  result = sum_i [ sum_k exp(T[i,k]) * M2[i,k] ] / [ sum_k exp(T[i,k]) ]
  T  = (A*w3 + a2) @ B.T,   M2 = (C1 + A*C2) @ B.T
so 8 cores run ONE program on different operands:
  cores 0-3 (A-group, mention shard I of 256): A=u_p[I], B=u_c,
      C1=W2[I], C2=W3[I], E=W1[I], a2=w2   (also computes term1 shard)
  cores 4-7 (B-group, candidate shard J):     A=u_c[J], B=u_p,
      C1=0,     C2=W4[J], E=0,     a2=w1
All operands are fed pre-transposed ([d, rows]) so the d-contraction lands on
the SBUF partition axis, then packed p-major ([128, chunks*cols]) so every
DMA is one contiguous 4-16 KB run per partition.  Matmuls run in float32r
(FP22 multiply, fp32 accumulate).  Per-core outputs are tiny partial-sum
vectors; the host reduces them and applies bias+relu.
"""

import sys

sys.path.insert(0, "/opt/trn_rl_repo")

import numpy as np

import concourse.bass as bass
import concourse.tile as tile
from concourse import mybir
from concourse.bass_utils import run_bass_kernel_spmd

F32 = mybir.dt.float32
F32R = mybir.dt.float32r

M = 1024  # mentions
N = 1024  # candidates
D = 2048  # feature dim (contraction)
NCORES = 8
ISH = 256  # per-core shard rows (A rows)
CH = D // 128  # 16 contraction chunks
ITILES = ISH // 128  # 2
KH = 512  # rhs free-dim per matmul (fp32 moving-operand max)
NKH = N // KH  # 2

# ---------------------------------------------------------------------------
# Workaround: the pinned neuronxcc walrus accepts fewer sync waits per
# instruction than Tile's semaphore assignment attaches (TensorScalar holds 1,
# the end-of-kernel Drain got 3, ...).  After scheduling, hoist excess waits
# of any over-capacity instruction onto same-engine EventSemaphores inserted
# right before it; each engine executes its stream in order, so the waits
# still gate the instruction.
_DEFAULT_CAP = 1
_WAIT_CAPS = {
    "InstTensorScalarPtr": 1,
    "InstTensorScalar": 1,
    "InstScalarTensorTensor": 1,
    "InstTensorReduce": 1,
}
_wfix_counter = [0]


def _legalize_waits(nc: bass.Bass) -> None:
    for f in nc.m.functions:
        for bb in f.blocks:
            il = bb.instructions
            out = []
            for inst in il:
                si = inst.sync_info
                waits = list(si.on_wait) if si and si.on_wait else []
                cap = _WAIT_CAPS.get(type(inst).__name__, _DEFAULT_CAP)
                if len(waits) > cap:
                    keep = waits[:cap]
                    for w in waits[cap:]:
                        _wfix_counter[0] += 1
                        out.append(
                            mybir.InstEventSemaphore(
                                name=f"I-wfix-{_wfix_counter[0]}",
                                engine=inst.engine,
                                ins=[],
                                outs=[],
                                sync_info=mybir.SyncInfo(on_wait=[w], on_update=[]),
                            )
                        )
                    inst.sync_info = mybir.SyncInfo(
                        on_wait=keep, on_update=list(si.on_update or [])
                    )
                out.append(inst)
            bb.instructions = out


# ---------------------------------------------------------------------------
def _emit(nc: bass.Bass, tc: tile.TileContext, io: dict) -> None:
    mult = mybir.AluOpType.mult
    add = mybir.AluOpType.add

    # p-major packed DRAM views: [128, CH * x] -> [128 parts, CH, x]
    at_r = io["at"].ap().rearrange("p (c i) -> p c i", c=CH)
    bt_r = io["bt"].ap().rearrange("p (c k) -> p c k", c=CH)
    c1_r = io["c1t"].ap().rearrange("p (c i) -> p c i", c=CH)
    c2_r = io["c2t"].ap().rearrange("p (c i) -> p c i", c=CH)
    et_r = io["et"].ap().rearrange("p (c i) -> p c i", c=CH)
    wv_r = io["wv"].ap().rearrange("p (c v) -> p c v", c=CH)

    import contextlib

    ctx = contextlib.ExitStack()
    singles = ctx.enter_context(tc.tile_pool(name="singles", bufs=1))
    scratch = ctx.enter_context(tc.tile_pool(name="scratch", bufs=3))
    psum = ctx.enter_context(tc.tile_pool(name="psum", bufs=4, space="PSUM"))

    at_sb = singles.tile([128, CH, ISH], F32)
    bt_sb = singles.tile([128, CH, N], F32R)
    c1_sb = singles.tile([128, CH, ISH], F32)
    c2_sb = singles.tile([128, CH, ISH], F32)
    et_sb = singles.tile([128, CH, ISH], F32)
    wv_sb = singles.tile([128, CH, 2], F32)
    asp_sb = singles.tile([128, CH, ISH], F32R)
    ct_sb = singles.tile([128, CH, ISH], F32R)

    zp = singles.tile([128, ITILES * NKH], F32)
    gp = singles.tile([128, ITILES * NKH], F32)
    ae_sb = singles.tile([128, CH], F32)
    gz_sb = singles.tile([128, ITILES], F32)

    # Input DMAs: everything is one contiguous run per partition.  bt (8 MB)
    # is issued from the Scalar HWDGE queue in quarters so matmuls can start
    # at ~25% loaded; the rest issue from Sync.
    nc.sync.dma_start(out=wv_sb, in_=wv_r)
    QC = 4  # chunks per bt DMA
    for q in range(CH // QC):
        nc.scalar.dma_start(
            out=bt_sb[:, q * QC : (q + 1) * QC, :], in_=bt_r[:, q * QC : (q + 1) * QC, :]
        )
    for h_ in range(2):
        sl = slice(h_ * (CH // 2), (h_ + 1) * (CH // 2))
        nc.sync.dma_start(out=at_sb[:, sl, :], in_=at_r[:, sl, :])
    nc.sync.dma_start(out=c2_sb, in_=c2_r)
    nc.sync.dma_start(out=c1_sb, in_=c1_r)
    nc.sync.dma_start(out=et_sb, in_=et_r)

    for c in range(CH):
        # ASp = A*w3 + a2   (folds the surviving softmax column-offset)
        nc.vector.tensor_scalar(
            out=asp_sb[:, c, :],
            in0=at_sb[:, c, :],
            scalar1=wv_sb[:, c, 0:1],
            scalar2=wv_sb[:, c, 1:2],
            op0=mult,
            op1=add,
        )
        # C = C1 + A*C2  (mult on DVE, add on the otherwise-idle GpSimd)
        nc.vector.tensor_tensor(
            out=ct_sb[:, c, :], in0=at_sb[:, c, :], in1=c2_sb[:, c, :], op=mult
        )
        nc.gpsimd.tensor_tensor(
            out=ct_sb[:, c, :], in0=ct_sb[:, c, :], in1=c1_sb[:, c, :], op=add
        )
        # term1 partials: ae[:, c] = rowsum(A*E), fused via accum_out
        ae_tmp = scratch.tile([128, ISH], F32, tag="ae_tmp")
        nc.vector.scalar_tensor_tensor(
            out=ae_tmp,
            in0=at_sb[:, c, :],
            scalar=1.0,
            in1=et_sb[:, c, :],
            op0=mult,
            op1=mult,
            accum_out=ae_sb[:, c : c + 1],
        )

    # Main contraction: T and M2 accumulate over CH chunks in PSUM,
    # then exp+rowsum (ACT) and mul+rowsum (DVE) fold k away.
    for it in range(ITILES):
        for kh in range(NKH):
            tps = psum.tile([128, KH], F32, tag="tps")
            mps = psum.tile([128, KH], F32, tag="mps")
            for c in range(CH):
                lt = asp_sb[:, c, it * 128 : (it + 1) * 128]
                lm = ct_sb[:, c, it * 128 : (it + 1) * 128]
                rhs = bt_sb[:, c, kh * KH : (kh + 1) * KH]
                nc.tensor.matmul(
                    tps, lhsT=lt, rhs=rhs, start=(c == 0), stop=(c == CH - 1)
                )
                nc.tensor.matmul(
                    mps, lhsT=lm, rhs=rhs, start=(c == 0), stop=(c == CH - 1)
                )
            col = it * NKH + kh
            ep = scratch.tile([128, KH], F32, tag="ep")
            nc.scalar.activation(
                out=ep,
                in_=tps,
                func=mybir.ActivationFunctionType.Exp,
                accum_out=zp[:, col : col + 1],
            )
            h = scratch.tile([128, KH], F32, tag="h")
            nc.vector.scalar_tensor_tensor(
                out=h,
                in0=ep,
                scalar=1.0,
                in1=mps,
                op0=mult,
                op1=mult,
                accum_out=gp[:, col : col + 1],
            )

    # gz[:, it] = (sum_kh G) / (sum_kh Z)
    zs = singles.tile([128, ITILES], F32)
    gs = singles.tile([128, ITILES], F32)
    for it in range(ITILES):
        nc.vector.tensor_tensor(
            out=zs[:, it : it + 1],
            in0=zp[:, it * NKH : it * NKH + 1],
            in1=zp[:, it * NKH + 1 : it * NKH + 2],
            op=add,
        )
        nc.vector.tensor_tensor(
            out=gs[:, it : it + 1],
            in0=gp[:, it * NKH : it * NKH + 1],
            in1=gp[:, it * NKH + 1 : it * NKH + 2],
            op=add,
        )
    nc.vector.reciprocal(out=zs, in_=zs)
    nc.vector.tensor_tensor(out=gz_sb, in0=gs, in1=zs, op=mult)

    nc.gpsimd.dma_start(out=io["out_gz"].ap(), in_=gz_sb)
    nc.gpsimd.dma_start(out=io["out_ae"].ap(), in_=ae_sb)
    ctx.close()


def _build() -> bass.Bass:
    nc = bass.Bass()
    io = {}
    io["at"] = nc.declare_dram_parameter("at", [128, CH * ISH], F32, isOutput=False)
    io["bt"] = nc.declare_dram_parameter("bt", [128, CH * N], F32R, isOutput=False)
    io["c1t"] = nc.declare_dram_parameter("c1t", [128, CH * ISH], F32, isOutput=False)
    io["c2t"] = nc.declare_dram_parameter("c2t", [128, CH * ISH], F32, isOutput=False)
    io["et"] = nc.declare_dram_parameter("et", [128, CH * ISH], F32, isOutput=False)
    io["wv"] = nc.declare_dram_parameter("wv", [128, CH * 2], F32, isOutput=False)
    io["out_gz"] = nc.declare_dram_parameter("out_gz", [128, ITILES], F32, isOutput=True)
    io["out_ae"] = nc.declare_dram_parameter("out_ae", [128, CH], F32, isOutput=True)
    with tile.TileContext(nc) as tc:
        _emit(nc, tc, io)
    _legalize_waits(nc)
    return nc


_NC_CACHE: bass.Bass | None = None


def _get_nc() -> bass.Bass:
    global _NC_CACHE
    if _NC_CACHE is None:
        _NC_CACHE = _build()
    return _NC_CACHE


def _pack(a2d: np.ndarray) -> np.ndarray:
    """[D, x] (d-major) -> [128, CH*x] p-major so each partition's data is
    one contiguous DRAM run."""
    x = a2d.shape[1]
    return np.ascontiguousarray(
        a2d.reshape(CH, 128, x).transpose(1, 0, 2).reshape(128, CH * x)
    )


def _in_maps(u_p, u_c, w_a, ffn_w):
    u_pT = np.ascontiguousarray(u_p.T)
    u_cT = np.ascontiguousarray(u_c.T)
    W = ffn_w.reshape(N, 4, D)
    W1T = np.ascontiguousarray(W[:, 0, :].T)
    W2T = np.ascontiguousarray(W[:, 1, :].T)
    W3T = np.ascontiguousarray(W[:, 2, :].T)
    W4T = np.ascontiguousarray(W[:, 3, :].T)
    wa = w_a[0]
    w1, w2, w3 = wa[:D], wa[D : 2 * D], wa[2 * D :]
    wv_a = _pack(np.ascontiguousarray(np.stack([w3, w2], axis=1)))
    wv_b = _pack(np.ascontiguousarray(np.stack([w3, w1], axis=1)))
    zeros = np.zeros((128, CH * ISH), np.float32)
    bt_a = _pack(u_cT)
    bt_b = _pack(u_pT)

    maps = []
    for ci in range(4):
        sl = slice(ISH * ci, ISH * (ci + 1))
        maps.append(
            {
                "at": _pack(u_pT[:, sl]),
                "bt": bt_a,
                "c1t": _pack(W2T[:, sl]),
                "c2t": _pack(W3T[:, sl]),
                "et": _pack(W1T[:, sl]),
                "wv": wv_a,
            }
        )
    for ci in range(4):
        sl = slice(ISH * ci, ISH * (ci + 1))
        maps.append(
            {
                "at": _pack(u_cT[:, sl]),
                "bt": bt_b,
                "c1t": zeros,
                "c2t": _pack(W4T[:, sl]),
                "et": zeros,
                "wv": wv_b,
            }
        )
    return maps


def kernel(u_p, u_c, w_a, ffn_w, ffn_b, **run_kwargs):
    nc = _get_nc()
    maps = _in_maps(
        np.asarray(u_p, np.float32),
        np.asarray(u_c, np.float32),
        np.asarray(w_a, np.float32),
        np.asarray(ffn_w, np.float32),
    )
    res = run_bass_kernel_spmd(nc, maps, core_ids=list(range(NCORES)), **run_kwargs)
    total = 0.0
    for r in res.results:
        total += r["out_gz"].sum(dtype=np.float64)
        total += r["out_ae"].sum(dtype=np.float64)
    f = np.float32(max(total + float(np.asarray(ffn_b)[0]), 0.0))
    out = np.array([f], dtype=np.float32)
    if run_kwargs:
        return out, res
    return out



# revision 3
# speedup vs baseline: 2.2794x; 2.2794x over previous
"""Trainium2 Bass kernel for nn_CrossAttentionLayer (m=n=1024, d=2048).

Math:  f = relu(term1 + term23 + term4 + ffn_b), where with
W1..W4 = ffn_w.reshape(n, 4, d) per-candidate blocks:
  term1  = sum_i u_p[i] . W1[i]                      (host: tiny scalar dot)
  term23 = <softmax_rows(S),    (W2 + u_p*W3) @ u_c.T>_F     (S = [m,n] logits)
  term4  = <softmax_rows(S.T),  (u_c*W4)      @ u_p.T>_F
Row-constant offsets cancel inside row-softmax, and the remaining column
offset folds into the matmul:  softmax_k(S[i,:]) = softmax_k((u_p[i]*w3 + w2) @ u_c.T).

Both inner products have the identical SPMD shape
  result = sum_i [ sum_k exp(T[i,k]) * M2[i,k] ] / [ sum_k exp(T[i,k]) ]
  T  = ASp @ B.T,   M2 = C @ B.T
with ASp = A*w3 + a2 and C = C1 + A*C2 folded on the host, so 8 cores run
ONE program on different operands:
  cores 0-3 (mention shard I of 256): A=u_p[I], B=u_c, C=(W2+u_p*W3)[I], a2=w2
  cores 4-7 (candidate shard J):      A=u_c[J], B=u_p, C=(u_c*W4)[J],    a2=w1
Operands are pre-transposed ([d, rows]), quantized to fp8e4m3 with static
scales (SA for ASp, SC for C; exp() descales T via the activation's scale
input, the host descales gz by SC), and packed p-major [128, chunks*cols]
so every DMA is one contiguous multi-KB run per partition.  Matmuls run in
fp8 DoubleRow perf mode (2 contraction rows per partition per cycle).
Per-core outputs are tiny gz row-vectors; the host reduces them, adds
term1 + bias, and applies relu.
"""

import sys

sys.path.insert(0, "/opt/trn_rl_repo")

import ml_dtypes
import numpy as np

import concourse.bass as bass
import concourse.tile as tile
from concourse import mybir
from concourse.bass_utils import run_bass_kernel_spmd

F32 = mybir.dt.float32
F8 = mybir.dt.float8e4
NP_F8 = ml_dtypes.float8_e4m3

M = 1024  # mentions
N = 1024  # candidates
D = 2048  # feature dim (contraction)
NCORES = 8
ISH = 256  # per-core shard rows (A rows)
CH = D // 128  # 16 contraction chunks of 128
ITILES = ISH // 128  # 2
KH = 512  # rhs free-dim per matmul (PSUM bank width in fp32)
NKH = N // KH  # 2

SA = 32.0  # ASp fp8 scale (entries ~0.03 sigma -> ~1)
SC = 512.0  # C fp8 scale (entries ~0.0015 sigma -> ~0.8)

# ---------------------------------------------------------------------------
# Workaround: the pinned neuronxcc walrus accepts fewer sync waits per
# instruction than Tile's semaphore assignment attaches.  After scheduling,
# hoist excess waits of any over-capacity instruction onto same-engine
# EventSemaphores inserted right before it; each engine executes its stream
# in order, so the waits still gate the instruction.
_DEFAULT_CAP = 1
_WAIT_CAPS = {
    "InstTensorScalarPtr": 1,
    "InstTensorScalar": 1,
    "InstScalarTensorTensor": 1,
    "InstTensorReduce": 1,
}
_wfix_counter = [0]


def _legalize_waits(nc: bass.Bass) -> None:
    for f in nc.m.functions:
        for bb in f.blocks:
            il = bb.instructions
            out = []
            for inst in il:
                si = inst.sync_info
                waits = list(si.on_wait) if si and si.on_wait else []
                cap = _WAIT_CAPS.get(type(inst).__name__, _DEFAULT_CAP)
                if len(waits) > cap:
                    keep = waits[:cap]
                    for w in waits[cap:]:
                        _wfix_counter[0] += 1
                        out.append(
                            mybir.InstEventSemaphore(
                                name=f"I-wfix-{_wfix_counter[0]}",
                                engine=inst.engine,
                                ins=[],
                                outs=[],
                                sync_info=mybir.SyncInfo(on_wait=[w], on_update=[]),
                            )
                        )
                    inst.sync_info = mybir.SyncInfo(
                        on_wait=keep, on_update=list(si.on_update or [])
                    )
                out.append(inst)
            bb.instructions = out


# ---------------------------------------------------------------------------
def _emit(nc: bass.Bass, tc: tile.TileContext, io: dict) -> None:
    mult = mybir.AluOpType.mult
    add = mybir.AluOpType.add
    DR = mybir.MatmulPerfMode.DoubleRow

    a_r = io["a8"].ap().rearrange("p (c i) -> p c i", c=CH)
    c_r = io["c8"].ap().rearrange("p (c i) -> p c i", c=CH)
    b_r = io["b8"].ap().rearrange("p (h c k) -> p h c k", h=NKH, c=CH)

    import contextlib

    ctx = contextlib.ExitStack()
    singles = ctx.enter_context(tc.tile_pool(name="singles", bufs=1))
    scratch = ctx.enter_context(tc.tile_pool(name="scratch", bufs=3))
    psum = ctx.enter_context(tc.tile_pool(name="psum", bufs=4, space="PSUM"))

    a_sb = singles.tile([128, CH, ISH], F8)
    c_sb = singles.tile([128, CH, ISH], F8)
    b_sb = singles.tile([128, NKH, CH, KH], F8)
    zp = singles.tile([128, ITILES * NKH], F32)
    gp = singles.tile([128, ITILES * NKH], F32)
    gz_sb = singles.tile([128, ITILES], F32)

    # Input DMAs, all on the Sync queue in consumption order: a8, the kh=0
    # half of b8 (in quarters so matmuls start early), c8, then kh=1.
    nc.sync.dma_start(out=a_sb, in_=a_r)
    QC = CH // 2  # 8 chunks per b8 piece = 4 KB per partition
    for h in range(NKH):
        for q in range(CH // QC):
            sl = slice(q * QC, (q + 1) * QC)
            nc.sync.dma_start(out=b_sb[:, h, sl, :], in_=b_r[:, h, sl, :])
        if h == 0:
            nc.sync.dma_start(out=c_sb, in_=c_r)

    # Main contraction: T and M2 accumulate over 8 DoubleRow chunks in PSUM,
    # then exp+rowsum (ACT, descaling by 1/SA) and mul+rowsum (DVE) fold k.
    for it in range(ITILES):
        isl = slice(it * 128, (it + 1) * 128)
        for kh in range(NKH):
            tps = psum.tile([128, KH], F32, tag="tps")
            mps = psum.tile([128, KH], F32, tag="mps")
            for c2 in range(CH // 2):
                sl2 = slice(2 * c2, 2 * c2 + 2)
                rhs = b_sb[:, kh, sl2, :]
                nc.tensor.matmul(
                    tps,
                    lhsT=a_sb[:, sl2, isl],
                    rhs=rhs,
                    start=(c2 == 0),
                    stop=(c2 == CH // 2 - 1),
                    perf_mode=DR,
                )
                nc.tensor.matmul(
                    mps,
                    lhsT=c_sb[:, sl2, isl],
                    rhs=rhs,
                    start=(c2 == 0),
                    stop=(c2 == CH // 2 - 1),
                    perf_mode=DR,
                )
            col = it * NKH + kh
            ep = scratch.tile([128, KH], F32, tag="ep")
            nc.scalar.activation(
                out=ep,
                in_=tps,
                func=mybir.ActivationFunctionType.Exp,
                scale=1.0 / SA,
                accum_out=zp[:, col : col + 1],
            )
            h2 = scratch.tile([128, KH], F32, tag="h")
            nc.vector.scalar_tensor_tensor(
                out=h2,
                in0=ep,
                scalar=1.0,
                in1=mps,
                op0=mult,
                op1=mult,
                accum_out=gp[:, col : col + 1],
            )

    # gz[:, it] = (sum_kh G) / (sum_kh Z)
    zs = singles.tile([128, ITILES], F32)
    gs = singles.tile([128, ITILES], F32)
    for it in range(ITILES):
        nc.vector.tensor_tensor(
            out=zs[:, it : it + 1],
            in0=zp[:, it * NKH : it * NKH + 1],
            in1=zp[:, it * NKH + 1 : it * NKH + 2],
            op=add,
        )
        nc.vector.tensor_tensor(
            out=gs[:, it : it + 1],
            in0=gp[:, it * NKH : it * NKH + 1],
            in1=gp[:, it * NKH + 1 : it * NKH + 2],
            op=add,
        )
    nc.vector.reciprocal(out=zs, in_=zs)
    nc.vector.tensor_tensor(out=gz_sb, in0=gs, in1=zs, op=mult)

    nc.gpsimd.dma_start(out=io["out_gz"].ap(), in_=gz_sb)
    ctx.close()


def _build() -> bass.Bass:
    nc = bass.Bass()
    io = {}
    io["a8"] = nc.declare_dram_parameter("a8", [128, CH * ISH], F8, isOutput=False)
    io["c8"] = nc.declare_dram_parameter("c8", [128, CH * ISH], F8, isOutput=False)
    io["b8"] = nc.declare_dram_parameter("b8", [128, NKH * CH * KH], F8, isOutput=False)
    io["out_gz"] = nc.declare_dram_parameter("out_gz", [128, ITILES], F32, isOutput=True)
    with tile.TileContext(nc) as tc:
        _emit(nc, tc, io)
    _legalize_waits(nc)
    return nc


_NC_CACHE: bass.Bass | None = None


def _get_nc() -> bass.Bass:
    global _NC_CACHE
    if _NC_CACHE is None:
        _NC_CACHE = _build()
    return _NC_CACHE


def _q8(a2d: np.ndarray, scale: float) -> np.ndarray:
    return np.clip(a2d * scale, -240.0, 240.0).astype(NP_F8)


def _pack(a2d: np.ndarray) -> np.ndarray:
    """[D, x] (d-major) -> [128, CH*x] p-major (chunk-major per partition)."""
    x = a2d.shape[1]
    return np.ascontiguousarray(
        a2d.reshape(CH, 128, x).transpose(1, 0, 2).reshape(128, CH * x)
    )


def _pack_b(b2d: np.ndarray) -> np.ndarray:
    """[D, N] -> [128, NKH*CH*KH] with per-partition layout [kh][chunk][col]."""
    return np.ascontiguousarray(
        b2d.reshape(CH, 128, NKH, KH).transpose(1, 2, 0, 3).reshape(128, NKH * CH * KH)
    )


def _in_maps(u_p, u_c, w_a, ffn_w):
    u_pT = np.ascontiguousarray(u_p.T)
    u_cT = np.ascontiguousarray(u_c.T)
    W = ffn_w.reshape(N, 4, D)
    wa = w_a[0]
    w1, w2, w3 = wa[:D], wa[D : 2 * D], wa[2 * D :]

    # host-folded operands, [d, rows]
    asp_a = u_pT * w3[:, None] + w2[:, None]
    asp_b = u_cT * w3[:, None] + w1[:, None]
    c_a = W[:, 1, :].T + u_pT * W[:, 2, :].T  # W2 + u_p*W3
    c_b = u_cT * W[:, 3, :].T  # u_c*W4

    b8_a = _pack_b(_q8(u_cT, 1.0))
    b8_b = _pack_b(_q8(u_pT, 1.0))

    maps = []
    for grp, (asp, cc, b8) in enumerate(((asp_a, c_a, b8_a), (asp_b, c_b, b8_b))):
        for ci in range(4):
            sl = slice(ISH * ci, ISH * (ci + 1))
            maps.append(
                {
                    "a8": _pack(_q8(asp[:, sl], SA)),
                    "c8": _pack(_q8(cc[:, sl], SC)),
                    "b8": b8,
                }
            )
    return maps


def kernel(u_p, u_c, w_a, ffn_w, ffn_b, **run_kwargs):
    nc = _get_nc()
    u_p = np.asarray(u_p, np.float32)
    u_c = np.asarray(u_c, np.float32)
    w_a = np.asarray(w_a, np.float32)
    ffn_w = np.asarray(ffn_w, np.float32)
    maps = _in_maps(u_p, u_c, w_a, ffn_w)
    res = run_bass_kernel_spmd(nc, maps, core_ids=list(range(NCORES)), **run_kwargs)
    total = 0.0
    for r in res.results:
        total += r["out_gz"].sum(dtype=np.float64)
    total /= SC
    # term1 = sum_j u_p[j] . W1[j] -- the scalar part of the final reduction
    total += float(
        np.einsum("ij,ij->", u_p, ffn_w.reshape(N, 4, D)[:, 0, :], dtype=np.float64)
    )
    f = np.float32(max(total + float(np.asarray(ffn_b)[0]), 0.0))
    out = np.array([f], dtype=np.float32)
    if run_kwargs:
        return out, res
    return out


# revision 10
# speedup vs baseline: 2.3490x; 1.0305x over previous
"""Trainium2 Bass kernel for nn_CrossAttentionLayer (m=n=1024, d=2048).

Math:  f = relu(term1 + term23 + term4 + ffn_b), where with
W1..W4 = ffn_w.reshape(n, 4, d) per-candidate blocks:
  term1  = sum_i u_p[i] . W1[i]                      (host: tiny scalar dot)
  term23 = <softmax_rows(S),    (W2 + u_p*W3) @ u_c.T>_F     (S = [m,n] logits)
  term4  = <softmax_rows(S.T),  (u_c*W4)      @ u_p.T>_F
Row-constant offsets cancel inside row-softmax, and the remaining column
offset folds into the matmul:  softmax_k(S[i,:]) = softmax_k((u_p[i]*w3 + w2) @ u_c.T).

Both inner products have the identical SPMD shape
  result = sum_i [ sum_k exp(T[i,k]) * M2[i,k] ] / [ sum_k exp(T[i,k]) ]
  T  = ASp @ B.T,   M2 = C @ B.T
with ASp = A*w3 + a2 and C = C1 + A*C2 folded on the host, so 8 cores run
ONE program on different operands:
  cores 0-3 (mention shard I of 256): A=u_p[I], B=u_c, C=(W2+u_p*W3)[I], a2=w2
  cores 4-7 (candidate shard J):      A=u_c[J], B=u_p, C=(u_c*W4)[J],    a2=w1
Operands are pre-transposed ([d, rows]), quantized to fp8e4m3 with static
scales (SA for ASp, SC for C; exp() descales T via the activation's scale
input, the host descales gz by SC), and packed p-major [128, chunks*cols]
so every DMA is one contiguous multi-KB run per partition.  Matmuls run in
fp8 DoubleRow perf mode (2 contraction rows per partition per cycle).
Per-core outputs are tiny gz row-vectors; the host reduces them, adds
term1 + bias, and applies relu.
"""

import sys

sys.path.insert(0, "/opt/trn_rl_repo")

import ml_dtypes
import numpy as np

import concourse.bass as bass
import concourse.tile as tile
from concourse import mybir
from concourse.bass_utils import run_bass_kernel_spmd

F32 = mybir.dt.float32
F8 = mybir.dt.float8e4
NP_F8 = ml_dtypes.float8_e4m3

M = 1024  # mentions
N = 1024  # candidates
D = 2048  # feature dim (contraction)
NCORES = 8
ISH = 256  # per-core shard rows (A rows)
CH = D // 128  # 16 contraction chunks of 128
ITILES = ISH // 128  # 2
KH = 512  # rhs free-dim per matmul (PSUM bank width in fp32)
NKH = N // KH  # 2

SA = 32.0  # ASp fp8 scale (entries ~0.03 sigma -> ~1)
SC = 512.0  # C fp8 scale (entries ~0.0015 sigma -> ~0.8)

# ---------------------------------------------------------------------------
# Workaround: the pinned neuronxcc walrus accepts fewer sync waits per
# instruction than Tile's semaphore assignment attaches.  After scheduling,
# hoist excess waits of any over-capacity instruction onto same-engine
# EventSemaphores inserted right before it; each engine executes its stream
# in order, so the waits still gate the instruction.
_DEFAULT_CAP = 1
_WAIT_CAPS = {
    "InstTensorScalarPtr": 1,
    "InstTensorScalar": 1,
    "InstScalarTensorTensor": 1,
    "InstTensorReduce": 1,
}
_wfix_counter = [0]


def _legalize_waits(nc: bass.Bass) -> None:
    for f in nc.m.functions:
        for bb in f.blocks:
            il = bb.instructions
            out = []
            for inst in il:
                si = inst.sync_info
                waits = list(si.on_wait) if si and si.on_wait else []
                cap = _WAIT_CAPS.get(type(inst).__name__, _DEFAULT_CAP)
                if len(waits) > cap:
                    keep = waits[:cap]
                    for w in waits[cap:]:
                        _wfix_counter[0] += 1
                        out.append(
                            mybir.InstEventSemaphore(
                                name=f"I-wfix-{_wfix_counter[0]}",
                                engine=inst.engine,
                                ins=[],
                                outs=[],
                                sync_info=mybir.SyncInfo(on_wait=[w], on_update=[]),
                            )
                        )
                    inst.sync_info = mybir.SyncInfo(
                        on_wait=keep, on_update=list(si.on_update or [])
                    )
                out.append(inst)
            bb.instructions = out


# ---------------------------------------------------------------------------
def _emit(nc: bass.Bass, tc: tile.TileContext, io: dict) -> None:
    mult = mybir.AluOpType.mult
    add = mybir.AluOpType.add
    DR = mybir.MatmulPerfMode.DoubleRow

    a_r = io["a8"].ap().rearrange("p (c i) -> p c i", c=CH)
    c_r = io["c8"].ap().rearrange("p (c i) -> p c i", c=CH)
    b_r = io["b8"].ap().rearrange("p (h c k) -> p h c k", h=NKH, c=CH)
    zg_r = io["out_zg"].ap().rearrange("p (c z) -> p c z", z=2)

    import contextlib

    ctx = contextlib.ExitStack()
    singles = ctx.enter_context(tc.tile_pool(name="singles", bufs=1))
    scratch = ctx.enter_context(tc.tile_pool(name="scratch", bufs=3))
    psum = ctx.enter_context(tc.tile_pool(name="psum", bufs=3, space="PSUM"))
    wpsum = ctx.enter_context(tc.tile_pool(name="wpsum", bufs=1, space="PSUM"))

    a_sb = singles.tile([128, CH, ISH], F8)
    c_sb = singles.tile([128, CH, ISH], F8)
    b_sb = singles.tile([128, NKH, CH, KH], F8)
    # zg[:, col, 0] = Z partials, zg[:, col, 1] = G partials; host divides.
    zg = singles.tile([128, ITILES * NKH, 2], F32)

    # Tensor-engine warmup: the PE array boots in a half-speed p-state and
    # only reaches full clock after ~3 us of sustained execution.  Burn that
    # ramp on junk matmuls while the input DMAs are still in flight.
    wa_sb = singles.tile([128, 2, 128], F8)
    wb_sb = singles.tile([128, 2, KH], F8)
    nc.gpsimd.memset(wa_sb, 0.0)
    nc.gpsimd.memset(wb_sb, 0.0)
    NWARM = 10
    for w in range(NWARM):
        wps = wpsum.tile([128, KH], F32, tag="wps")
        nc.tensor.matmul(wps, lhsT=wa_sb, rhs=wb_sb, perf_mode=DR)

    # Input DMAs in consumption order, split across two queues so descriptor
    # issue (~0.6 us each) isn't serialized on one engine:
    #   sync:   a8/c8 halves interleaved (stationary operands, needed first)
    #   vector: b8 kh=0 in quarters (moving operand), then kh=1
    hc = CH // 2
    nc.sync.dma_start(out=a_sb[:, :hc, :], in_=a_r[:, :hc, :])
    nc.sync.dma_start(out=c_sb[:, :hc, :], in_=c_r[:, :hc, :])
    nc.sync.dma_start(out=a_sb[:, hc:, :], in_=a_r[:, hc:, :])
    nc.sync.dma_start(out=c_sb[:, hc:, :], in_=c_r[:, hc:, :])
    for q in range(2):
        sl = slice(q * hc, (q + 1) * hc)
        nc.gpsimd.dma_start(out=b_sb[:, 0, sl, :], in_=b_r[:, 0, sl, :])
    nc.gpsimd.dma_start(out=b_sb[:, 1, :, :], in_=b_r[:, 1, :, :])

    # Main contraction: T and M2 accumulate over 8 DoubleRow chunks in PSUM,
    # then exp+rowsum (ACT, descaling by 1/SA) and mul+rowsum (DVE) fold k
    # away.  Each group's Z/G partial columns stream out as they finish.
    for it in range(ITILES):
        isl = slice(it * 128, (it + 1) * 128)
        for kh in range(NKH):
            tps = psum.tile([128, KH], F32, tag="tps")
            mps = psum.tile([128, KH], F32, tag="mps")
            for c2 in range(CH // 2):
                sl2 = slice(2 * c2, 2 * c2 + 2)
                rhs = b_sb[:, kh, sl2, :]
                nc.tensor.matmul(
                    tps,
                    lhsT=a_sb[:, sl2, isl],
                    rhs=rhs,
                    start=(c2 == 0),
                    stop=(c2 == CH // 2 - 1),
                    perf_mode=DR,
                )
                nc.tensor.matmul(
                    mps,
                    lhsT=c_sb[:, sl2, isl],
                    rhs=rhs,
                    start=(c2 == 0),
                    stop=(c2 == CH // 2 - 1),
                    perf_mode=DR,
                )
            col = it * NKH + kh
            ep = scratch.tile([128, KH], F32, tag="ep")
            nc.scalar.activation(
                out=ep,
                in_=tps,
                func=mybir.ActivationFunctionType.Exp,
                scale=1.0 / SA,
                accum_out=zg[:, col, 0:1],
            )
            h2 = scratch.tile([128, KH], F32, tag="h")
            nc.vector.scalar_tensor_tensor(
                out=h2,
                in0=ep,
                scalar=1.0,
                in1=mps,
                op0=mult,
                op1=mult,
                accum_out=zg[:, col, 1:2],
            )
            nc.sync.dma_start(out=zg_r[:, col, :], in_=zg[:, col, :])
    ctx.close()


def _build() -> bass.Bass:
    nc = bass.Bass()
    io = {}
    io["a8"] = nc.declare_dram_parameter("a8", [128, CH * ISH], F8, isOutput=False)
    io["c8"] = nc.declare_dram_parameter("c8", [128, CH * ISH], F8, isOutput=False)
    io["b8"] = nc.declare_dram_parameter("b8", [128, NKH * CH * KH], F8, isOutput=False)
    io["out_zg"] = nc.declare_dram_parameter(
        "out_zg", [128, 2 * ITILES * NKH], F32, isOutput=True
    )
    with tile.TileContext(nc) as tc:
        _emit(nc, tc, io)
    _legalize_waits(nc)
    return nc


_NC_CACHE: bass.Bass | None = None


def _get_nc() -> bass.Bass:
    global _NC_CACHE
    if _NC_CACHE is None:
        _NC_CACHE = _build()
    return _NC_CACHE


def _q8(a2d: np.ndarray, scale: float) -> np.ndarray:
    return np.clip(a2d * scale, -240.0, 240.0).astype(NP_F8)


def _pack(a2d: np.ndarray) -> np.ndarray:
    """[D, x] (d-major) -> [128, CH*x] p-major (chunk-major per partition)."""
    x = a2d.shape[1]
    return np.ascontiguousarray(
        a2d.reshape(CH, 128, x).transpose(1, 0, 2).reshape(128, CH * x)
    )


def _pack_b(b2d: np.ndarray) -> np.ndarray:
    """[D, N] -> [128, NKH*CH*KH] with per-partition layout [kh][chunk][col]."""
    return np.ascontiguousarray(
        b2d.reshape(CH, 128, NKH, KH).transpose(1, 2, 0, 3).reshape(128, NKH * CH * KH)
    )


def _in_maps(u_p, u_c, w_a, ffn_w):
    u_pT = np.ascontiguousarray(u_p.T)
    u_cT = np.ascontiguousarray(u_c.T)
    W = ffn_w.reshape(N, 4, D)
    wa = w_a[0]
    w1, w2, w3 = wa[:D], wa[D : 2 * D], wa[2 * D :]

    # host-folded operands, [d, rows]
    asp_a = u_pT * w3[:, None] + w2[:, None]
    asp_b = u_cT * w3[:, None] + w1[:, None]
    c_a = W[:, 1, :].T + u_pT * W[:, 2, :].T  # W2 + u_p*W3
    c_b = u_cT * W[:, 3, :].T  # u_c*W4

    b8_a = _pack_b(_q8(u_cT, 1.0))
    b8_b = _pack_b(_q8(u_pT, 1.0))

    maps = []
    for grp, (asp, cc, b8) in enumerate(((asp_a, c_a, b8_a), (asp_b, c_b, b8_b))):
        for ci in range(4):
            sl = slice(ISH * ci, ISH * (ci + 1))
            maps.append(
                {
                    "a8": _pack(_q8(asp[:, sl], SA)),
                    "c8": _pack(_q8(cc[:, sl], SC)),
                    "b8": b8,
                }
            )
    return maps


def kernel(u_p, u_c, w_a, ffn_w, ffn_b, **run_kwargs):
    nc = _get_nc()
    u_p = np.asarray(u_p, np.float32)
    u_c = np.asarray(u_c, np.float32)
    w_a = np.asarray(w_a, np.float32)
    ffn_w = np.asarray(ffn_w, np.float32)
    maps = _in_maps(u_p, u_c, w_a, ffn_w)
    res = run_bass_kernel_spmd(nc, maps, core_ids=list(range(NCORES)), **run_kwargs)
    total = 0.0
    for r in res.results:
        zg = r["out_zg"].reshape(128, ITILES * NKH, 2).astype(np.float64)
        # row (it*128+p): z = sum_kh zg[p,it*NKH+kh,0], g likewise in [...,1]
        z = zg[:, :, 0].reshape(128, ITILES, NKH).sum(axis=2)
        g = zg[:, :, 1].reshape(128, ITILES, NKH).sum(axis=2)
        total += (g / z).sum(dtype=np.float64)
    total /= SC
    # term1 = sum_j u_p[j] . W1[j] -- the scalar part of the final reduction
    total += float(
        np.einsum("ij,ij->", u_p, ffn_w.reshape(N, 4, D)[:, 0, :], dtype=np.float64)
    )
    f = np.float32(max(total + float(np.asarray(ffn_b)[0]), 0.0))
    out = np.array([f], dtype=np.float32)
    if run_kwargs:
        return out, res
    return out


# revision 11
# speedup vs baseline: 2.4888x; 1.0595x over previous
"""Trainium2 Bass kernel for nn_CrossAttentionLayer (m=n=1024, d=2048).

Math:  f = relu(term1 + term23 + term4 + ffn_b), where with
W1..W4 = ffn_w.reshape(n, 4, d) per-candidate blocks:
  term1  = sum_i u_p[i] . W1[i]                      (host: tiny scalar dot)
  term23 = <softmax_rows(S),    (W2 + u_p*W3) @ u_c.T>_F     (S = [m,n] logits)
  term4  = <softmax_rows(S.T),  (u_c*W4)      @ u_p.T>_F
Row-constant offsets cancel inside row-softmax, and the remaining column
offset folds into the matmul:  softmax_k(S[i,:]) = softmax_k((u_p[i]*w3 + w2) @ u_c.T).

Both inner products have the identical SPMD shape
  result = sum_i [ sum_k exp(T[i,k]) * M2[i,k] ] / [ sum_k exp(T[i,k]) ]
  T  = ASp @ B.T,   M2 = C @ B.T
with ASp = A*w3 + a2 and C = C1 + A*C2 folded on the host, so 8 cores run
ONE program on different operands:
  cores 0-3 (mention shard I of 256): A=u_p[I], B=u_c, C=(W2+u_p*W3)[I], a2=w2
  cores 4-7 (candidate shard J):      A=u_c[J], B=u_p, C=(u_c*W4)[J],    a2=w1
Operands are pre-transposed ([d, rows]), quantized to fp8e4m3 with static
scales (SA for ASp, SC for C; exp() descales T via the activation's scale
input, the host descales gz by SC), and packed p-major [128, chunks*cols]
so every DMA is one contiguous multi-KB run per partition.  Matmuls run in
fp8 DoubleRow perf mode (2 contraction rows per partition per cycle).
Per-core outputs are tiny gz row-vectors; the host reduces them, adds
term1 + bias, and applies relu.
"""

import sys

sys.path.insert(0, "/opt/trn_rl_repo")

import ml_dtypes
import numpy as np

import concourse.bass as bass
import concourse.tile as tile
from concourse import mybir
from concourse.bass_utils import run_bass_kernel_spmd

F32 = mybir.dt.float32
F8 = mybir.dt.float8e4
NP_F8 = ml_dtypes.float8_e4m3

M = 1024  # mentions
N = 1024  # candidates
D = 2048  # feature dim (contraction)
NCORES = 8
ISH = 256  # per-core shard rows (A rows)
CH = D // 128  # 16 contraction chunks of 128
ITILES = ISH // 128  # 2
KH = 512  # rhs free-dim per matmul (PSUM bank width in fp32)
NKH = N // KH  # 2

SA = 32.0  # ASp fp8 scale (entries ~0.03 sigma -> ~1)
SC = 512.0  # C fp8 scale (entries ~0.0015 sigma -> ~0.8)

# ---------------------------------------------------------------------------
# Workaround: the pinned neuronxcc walrus accepts fewer sync waits per
# instruction than Tile's semaphore assignment attaches.  After scheduling,
# hoist excess waits of any over-capacity instruction onto same-engine
# EventSemaphores inserted right before it; each engine executes its stream
# in order, so the waits still gate the instruction.
_DEFAULT_CAP = 1
_WAIT_CAPS = {
    "InstTensorScalarPtr": 1,
    "InstTensorScalar": 1,
    "InstScalarTensorTensor": 1,
    "InstTensorReduce": 1,
}
_wfix_counter = [0]


def _legalize_waits(nc: bass.Bass) -> None:
    for f in nc.m.functions:
        for bb in f.blocks:
            il = bb.instructions
            out = []
            for inst in il:
                si = inst.sync_info
                waits = list(si.on_wait) if si and si.on_wait else []
                cap = _WAIT_CAPS.get(type(inst).__name__, _DEFAULT_CAP)
                if len(waits) > cap:
                    keep = waits[:cap]
                    for w in waits[cap:]:
                        _wfix_counter[0] += 1
                        out.append(
                            mybir.InstEventSemaphore(
                                name=f"I-wfix-{_wfix_counter[0]}",
                                engine=inst.engine,
                                ins=[],
                                outs=[],
                                sync_info=mybir.SyncInfo(on_wait=[w], on_update=[]),
                            )
                        )
                    inst.sync_info = mybir.SyncInfo(
                        on_wait=keep, on_update=list(si.on_update or [])
                    )
                out.append(inst)
            bb.instructions = out


# ---------------------------------------------------------------------------
def _emit(nc: bass.Bass, tc: tile.TileContext, io: dict) -> None:
    mult = mybir.AluOpType.mult
    add = mybir.AluOpType.add
    DR = mybir.MatmulPerfMode.DoubleRow

    a_r = io["a8"].ap().rearrange("p (c i) -> p c i", c=CH)
    c_r = io["c8"].ap().rearrange("p (c i) -> p c i", c=CH)
    b_r = io["b8"].ap().rearrange("p (h c k) -> p h c k", h=NKH, c=CH)
    zg_r = io["out_zg"].ap().rearrange("p (c z) -> p c z", z=2)

    import contextlib

    ctx = contextlib.ExitStack()
    singles = ctx.enter_context(tc.tile_pool(name="singles", bufs=1))
    scratch = ctx.enter_context(tc.tile_pool(name="scratch", bufs=3))
    psum = ctx.enter_context(tc.tile_pool(name="psum", bufs=4, space="PSUM"))

    a_sb = singles.tile([128, CH, ISH], F8)
    c_sb = singles.tile([128, CH, ISH], F8)
    b_sb = singles.tile([128, NKH, CH, KH], F8)
    # zg[:, col, 0] = Z partials, zg[:, col, 1] = G partials; host divides.
    zg = singles.tile([128, ITILES * NKH, 2], F32)

    # Tensor-engine warmup: the PE array boots in a half-speed p-state and
    # reaches full clock only after ~5 us of GAPLESS execution (idle gaps
    # reset the ramp).  Burn the DMA-wait window on one junk accumulation
    # chain -- accumulating matmuls pipeline back-to-back, so the ramp
    # carries straight into the real stream that follows on the engine.
    wa_sb = singles.tile([128, 2, 128], F8)
    wb_sb = singles.tile([128, 2, KH], F8)
    nc.vector.memset(wa_sb, 0.0)
    nc.vector.memset(wb_sb, 0.0)
    NWARM = 8
    wps = psum.tile([128, KH], F32, tag="tps")
    for w in range(NWARM):
        nc.tensor.matmul(
            wps, lhsT=wa_sb, rhs=wb_sb, start=(w == 0), stop=(w == NWARM - 1),
            perf_mode=DR,
        )

    # Input DMAs in consumption order, split across two queues so descriptor
    # issue (~0.6 us each) isn't serialized on one engine:
    #   sync:   a8/c8 halves interleaved (stationary operands, needed first)
    #   vector: b8 kh=0 in quarters (moving operand), then kh=1
    hc = CH // 2
    nc.sync.dma_start(out=a_sb[:, :hc, :], in_=a_r[:, :hc, :])
    nc.sync.dma_start(out=c_sb[:, :hc, :], in_=c_r[:, :hc, :])
    nc.sync.dma_start(out=a_sb[:, hc:, :], in_=a_r[:, hc:, :])
    nc.sync.dma_start(out=c_sb[:, hc:, :], in_=c_r[:, hc:, :])
    for q in range(2):
        sl = slice(q * hc, (q + 1) * hc)
        nc.gpsimd.dma_start(out=b_sb[:, 0, sl, :], in_=b_r[:, 0, sl, :])
    nc.gpsimd.dma_start(out=b_sb[:, 1, :, :], in_=b_r[:, 1, :, :])

    # Main contraction: T and M2 accumulate over 8 DoubleRow chunks in PSUM,
    # then exp+rowsum (ACT, descaling by 1/SA) and mul+rowsum (DVE) fold k
    # away.  Each group's Z/G partial columns stream out as they finish.
    for kh in range(NKH):
        for it in range(ITILES):
            isl = slice(it * 128, (it + 1) * 128)
            tps = psum.tile([128, KH], F32, tag="tps")
            mps = psum.tile([128, KH], F32, tag="mps")
            for c2 in range(CH // 2):
                sl2 = slice(2 * c2, 2 * c2 + 2)
                rhs = b_sb[:, kh, sl2, :]
                nc.tensor.matmul(
                    tps,
                    lhsT=a_sb[:, sl2, isl],
                    rhs=rhs,
                    start=(c2 == 0),
                    stop=(c2 == CH // 2 - 1),
                    perf_mode=DR,
                )
                nc.tensor.matmul(
                    mps,
                    lhsT=c_sb[:, sl2, isl],
                    rhs=rhs,
                    start=(c2 == 0),
                    stop=(c2 == CH // 2 - 1),
                    perf_mode=DR,
                )
            col = kh * ITILES + it
            ep = scratch.tile([128, KH], F32, tag="ep")
            nc.scalar.activation(
                out=ep,
                in_=tps,
                func=mybir.ActivationFunctionType.Exp,
                scale=1.0 / SA,
                accum_out=zg[:, col, 0:1],
            )
            h2 = scratch.tile([128, KH], F32, tag="h")
            nc.vector.scalar_tensor_tensor(
                out=h2,
                in0=ep,
                scalar=1.0,
                in1=mps,
                op0=mult,
                op1=mult,
                accum_out=zg[:, col, 1:2],
            )
            nc.sync.dma_start(out=zg_r[:, col, :], in_=zg[:, col, :])
    ctx.close()


def _build() -> bass.Bass:
    nc = bass.Bass()
    io = {}
    io["a8"] = nc.declare_dram_parameter("a8", [128, CH * ISH], F8, isOutput=False)
    io["c8"] = nc.declare_dram_parameter("c8", [128, CH * ISH], F8, isOutput=False)
    io["b8"] = nc.declare_dram_parameter("b8", [128, NKH * CH * KH], F8, isOutput=False)
    io["out_zg"] = nc.declare_dram_parameter(
        "out_zg", [128, 2 * ITILES * NKH], F32, isOutput=True
    )
    with tile.TileContext(nc) as tc:
        _emit(nc, tc, io)
    _legalize_waits(nc)
    return nc


_NC_CACHE: bass.Bass | None = None


def _get_nc() -> bass.Bass:
    global _NC_CACHE
    if _NC_CACHE is None:
        _NC_CACHE = _build()
    return _NC_CACHE


def _q8(a2d: np.ndarray, scale: float) -> np.ndarray:
    return np.clip(a2d * scale, -240.0, 240.0).astype(NP_F8)


def _pack(a2d: np.ndarray) -> np.ndarray:
    """[D, x] (d-major) -> [128, CH*x] p-major (chunk-major per partition)."""
    x = a2d.shape[1]
    return np.ascontiguousarray(
        a2d.reshape(CH, 128, x).transpose(1, 0, 2).reshape(128, CH * x)
    )


def _pack_b(b2d: np.ndarray) -> np.ndarray:
    """[D, N] -> [128, NKH*CH*KH] with per-partition layout [kh][chunk][col]."""
    return np.ascontiguousarray(
        b2d.reshape(CH, 128, NKH, KH).transpose(1, 2, 0, 3).reshape(128, NKH * CH * KH)
    )


def _in_maps(u_p, u_c, w_a, ffn_w):
    u_pT = np.ascontiguousarray(u_p.T)
    u_cT = np.ascontiguousarray(u_c.T)
    W = ffn_w.reshape(N, 4, D)
    wa = w_a[0]
    w1, w2, w3 = wa[:D], wa[D : 2 * D], wa[2 * D :]

    # host-folded operands, [d, rows]
    asp_a = u_pT * w3[:, None] + w2[:, None]
    asp_b = u_cT * w3[:, None] + w1[:, None]
    c_a = W[:, 1, :].T + u_pT * W[:, 2, :].T  # W2 + u_p*W3
    c_b = u_cT * W[:, 3, :].T  # u_c*W4

    b8_a = _pack_b(_q8(u_cT, 1.0))
    b8_b = _pack_b(_q8(u_pT, 1.0))

    maps = []
    for grp, (asp, cc, b8) in enumerate(((asp_a, c_a, b8_a), (asp_b, c_b, b8_b))):
        for ci in range(4):
            sl = slice(ISH * ci, ISH * (ci + 1))
            maps.append(
                {
                    "a8": _pack(_q8(asp[:, sl], SA)),
                    "c8": _pack(_q8(cc[:, sl], SC)),
                    "b8": b8,
                }
            )
    return maps


def kernel(u_p, u_c, w_a, ffn_w, ffn_b, **run_kwargs):
    nc = _get_nc()
    u_p = np.asarray(u_p, np.float32)
    u_c = np.asarray(u_c, np.float32)
    w_a = np.asarray(w_a, np.float32)
    ffn_w = np.asarray(ffn_w, np.float32)
    maps = _in_maps(u_p, u_c, w_a, ffn_w)
    res = run_bass_kernel_spmd(nc, maps, core_ids=list(range(NCORES)), **run_kwargs)
    total = 0.0
    for r in res.results:
        zg = r["out_zg"].reshape(128, NKH, ITILES, 2).astype(np.float64)
        # row (it*128+p): z = sum_kh zg[p,kh,it,0], g likewise in [...,1]
        z = zg[:, :, :, 0].sum(axis=1)
        g = zg[:, :, :, 1].sum(axis=1)
        total += (g / z).sum(dtype=np.float64)
    total /= SC
    # term1 = sum_j u_p[j] . W1[j] -- the scalar part of the final reduction
    total += float(
        np.einsum("ij,ij->", u_p, ffn_w.reshape(N, 4, D)[:, 0, :], dtype=np.float64)
    )
    f = np.float32(max(total + float(np.asarray(ffn_b)[0]), 0.0))
    out = np.array([f], dtype=np.float32)
    if run_kwargs:
        return out, res
    return out
